# revision 28
# baseline (speedup 1.0000x reference)
"""Trainium2 Bass kernel for nn_DistillingLayer: per-channel shared-weight
Conv1d(k=3, stride=2, pad=1) + ELU + MaxPool1d(k=3, stride=2, pad=1) over
x:(16, 4096, 512) f32.

Strategy
--------
- Data-parallel over batch: 8 cores x 2 batches each. No communication.
- Layout: L lives in the SBUF *free* dimension. Each partition owns S=16
  consecutive L-rows (times D=512 channels) plus a 3-row halo, so the whole
  conv+pool dataflow is per-partition local (DVE lanes cannot cross
  partitions).
- ELU is monotonic, so maxpool commutes with it: pool the *pre-activation*
  conv outputs, then apply ELU once on the pooled result (half the L-rows).
  The conv bias folds into the conv pass itself.
- Conv outputs are computed de-interleaved (ce[m]=c[2m], co[m]=c[2m+1]) so
  every elementwise op runs on contiguous or simply-strided views:
      c[i] = w0*x[2i-1] + w1*x[2i] + w2*x[2i+1] + b
      out[m] = elu(max(co[m-1], ce[m], co[m]))
- ELU(v) = max(v, exp(min(v, 0)) - 1); min via relu(-v) on ScalarE with
  scale=-1, exp on ScalarE, final fused (e-1) max v on VectorE
  scalar_tensor_tensor.
- Weights/bias are baked as immediates (kernel() receives concrete values).
"""

import os
import sys

import numpy as np

for _p in ("/opt/trn_rl_repo", "/root/.axon_site/_ro/trn_rl_repo"):
    if os.path.isdir(_p) and _p not in sys.path:
        sys.path.append(_p)

import json as _json

import concourse.bass as bass
import concourse.bass2jax as bass2jax
import concourse.bass_utils as bass_utils
import concourse.mybir as mybir
from concourse.bass_utils import run_bass_kernel_spmd
from concourse.tile import TileContext

# ---------------------------------------------------------------------------
# Workaround: this container's walrus build rejects instructions carrying more
# than one sync wait ("Too many sync wait commands" in setupSyncWait). Tile's
# scheduler freely attaches several waits to one instruction, so post-process
# the BIR JSON before compile: hoist all but the last wait onto same-engine
# NoOps inserted just before the instruction (per-engine program order makes
# sequential waits equivalent to a multi-wait).
# ---------------------------------------------------------------------------

_orig_compile_bir_kernel = bass_utils.compile_bir_kernel


def _split_multi_waits(bir_json: bytes) -> bytes:
    j = _json.loads(bir_json)
    ctr = 0
    changed = False
    for fn in j["functions"]:
        for bb in fn["blocks"]:
            out = []
            for ins in bb["instructions"]:
                si = ins.get("sync_info")
                waits = (si.get("on_wait") or []) if si else []
                if len(waits) > 1:
                    changed = True
                    for w in waits[:-1]:
                        ctr += 1
                        out.append(
                            {
                                "debug": ins.get("debug", 0),
                                "engine": ins["engine"],
                                "ins": [],
                                "outs": [],
                                "name": f"waitsplit-{ctr}",
                                "opcode": "NoOp",
                                "text_hint": "waitsplit",
                                "sync_info": {"on_update": [], "on_wait": [w]},
                            }
                        )
                    si["on_wait"] = [waits[-1]]
                out.append(ins)
            bb["instructions"] = out
    if not changed:
        return bir_json
    return _json.dumps(j).encode()


def _patched_compile_bir_kernel(bir_json, tmpdir, neff_name="file.neff"):
    return _orig_compile_bir_kernel(_split_multi_waits(bir_json), tmpdir, neff_name)


bass_utils.compile_bir_kernel = _patched_compile_bir_kernel
bass2jax.compile_bir_kernel = _patched_compile_bir_kernel

# Event-semaphore barriers cost ~3us each on cayman; the sequencer-level
# sem-only variant is ~10x cheaper. All barrier uses here (const-AP preamble,
# TileContext exit) already wait on the relevant completion semaphores, so the
# per-engine drains the full barrier adds are redundant.
_orig_barrier = bass.Bass.all_engine_barrier


def _fast_barrier(self, *, sem_only=False):
    return _orig_barrier(self, sem_only=True)


bass.Bass.all_engine_barrier = _fast_barrier

# ---------------------------------------------------------------------------

N_CORES = 8
B, L, D = 16, 4096, 512
BPC = B // N_CORES  # batches per core
S = 16              # L-rows per partition per tile
T = L // (128 * S)  # tiles per batch
LC = L // 2         # conv output length
LP = LC // 2        # pool output length
JT = S // 4         # pool-output rows per partition per tile

F32 = mybir.dt.float32
ALU = mybir.AluOpType
AF = mybir.ActivationFunctionType

_cache: dict = {}

# Exposed for test harnesses: the BassKernelResults of the last run.
LAST_RESULT = None


def _build(w0: float, w1: float, w2: float, bias: float) -> bass.Bass:
    nc = bass.Bass()
    # x is host-padded with 3 zero rows at the front of L: padded row r
    # holds true row r-3. This makes every tile a uniform full-128-partition
    # DMA (SWDGE descriptor fan-out degenerates to 1-2 SDMA engines for
    # partition counts != 128) and provides the conv zero-padding for free.
    x = nc.dram_tensor("x", [BPC, L + 3, D], F32, kind="ExternalInput")
    y = nc.dram_tensor("y", [BPC, LP, D], F32, kind="ExternalOutput")

    xrow = D              # elements per L-row
    xbat = (L + 3) * D    # elements per (padded) input batch
    ybat = LP * D

    # Variable tile schedule per batch: small tiles at the global start
    # (fast pipeline fill) and end (short drain tail), big in the middle.
    # Each entry is (row_base, S_t); rows covered = 128 * S_t.
    sched0 = [(0, 16), (2048, 16)]
    sched1 = sched0

    with TileContext(nc) as tc:
        with (
            tc.tile_pool(name="xp", bufs=3) as xp,
            tc.tile_pool(name="yp", bufs=2) as yp,
            tc.tile_pool(name="pp", bufs=2) as pp,
            tc.tile_pool(name="rp", bufs=2) as rp,
        ):
            # Output DMAs are emitted one tile LATE on the gpsimd queue so
            # their wait-for-compute is already satisfied when the trigger
            # executes and never gates the next tile's input DMA trigger
            # (engine DMA queues execute waits in program order).
            pending_out = None
            tiles = [(b, rb, st)
                     for b in range(BPC)
                     for rb, st in (sched0 if b == 0 else sched1)]
            for b, row_base, St in tiles:
                    Q = St // 2 + 1   # conv rows per partition (with halo)
                    Jt = St // 4      # pool-output rows per partition
                    # X layout per partition: 3 halo rows then S own rows.
                    # HBM supplies only the S own rows (plus 6KB for
                    # partition 0's halo, which reads the host pad zeros at
                    # row_base=0); partitions 1..127 get their halo from
                    # partition p-1's last rows via two SBUF->SBUF shifted
                    # DMAs (64+63 partitions — 127-partition DMAs degenerate
                    # to one SDMA engine, <=64 fan out fine).
                    X = xp.tile([128, (St + 3) * D], F32)
                    nc.gpsimd.dma_start(
                        out=X[:, 3 * D :],
                        in_=bass.AP(
                            x,
                            b * xbat + (row_base + 3) * xrow,
                            [[St * xrow, 128], [1, St * xrow]],
                        ),
                    )
                    if pending_out is not None:
                        nc.gpsimd.dma_start(out=pending_out[0], in_=pending_out[1])
                        pending_out = None
                    nc.sync.dma_start(
                        out=X[0:1, 0 : 3 * D],
                        in_=bass.AP(
                            x,
                            b * xbat + row_base * xrow,
                            [[3 * xrow, 1], [1, 3 * xrow]],
                        ),
                    )
                    nc.sync.dma_start(
                        out=X[1:65, 0 : 3 * D],
                        in_=X[0:64, St * D : (St + 3) * D],
                    )
                    nc.sync.dma_start(
                        out=X[65:128, 0 : 3 * D],
                        in_=X[64:127, St * D : (St + 3) * D],
                    )

                    Xv = X[:, :].rearrange("p (r d) -> p r d", d=D)
                    # conv row q (local) = c[2*O0 - 1 + q]; taps are x rows
                    # (local) 2q, 2q+1, 2q+2
                    ya = Xv[:, 0 : St + 1 : 2, :]
                    yb = Xv[:, 1 : St + 2 : 2, :]
                    yc = Xv[:, 2 : St + 3 : 2, :]

                    Y = yp.tile([128, Q * D], F32)
                    y3 = Y[:, :].rearrange("p (q d) -> p q d", d=D)

                    # conv (bias folded in): c = w0*ya + w1*yb + w2*yc + bias
                    nc.scalar.activation(y3, ya, AF.Copy, bias=bias, scale=w0)
                    nc.vector.scalar_tensor_tensor(
                        y3, yb, w1, y3, op0=ALU.mult, op1=ALU.add
                    )
                    nc.vector.scalar_tensor_tensor(
                        y3, yc, w2, y3, op0=ALU.mult, op1=ALU.add
                    )
                    if row_base == 0:
                        # left pool pad: c[-1] = -inf (partition 0 only)
                        nc.vector.memset(Y[0:1, 0:D], float("-inf"))

                    # maxpool (pre-activation; ELU is monotonic):
                    # out[jl] = max(Y[2jl], Y[2jl+1], Y[2jl+2])
                    P = pp.tile([128, Jt * D], F32)
                    p3 = P[:, :].rearrange("p (j d) -> p j d", d=D)
                    nc.vector.tensor_tensor(
                        p3,
                        y3[:, 0 : 2 * Jt - 1 : 2, :],
                        y3[:, 1 : 2 * Jt : 2, :],
                        op=ALU.max,
                    )
                    nc.vector.tensor_tensor(
                        p3, p3, y3[:, 2 : 2 * Jt + 1 : 2, :], op=ALU.max
                    )

                    # ELU(v) = max(v, exp(min(v,0)) - 1)
                    R = rp.tile([128, Jt * D], F32)
                    nc.scalar.activation(R[:, :], P[:, :], AF.Relu, scale=-1.0)
                    nc.scalar.activation(R[:, :], R[:, :], AF.Exp, scale=-1.0)
                    nc.vector.scalar_tensor_tensor(
                        R[:, :], R[:, :], -1.0, P[:, :], op0=ALU.add, op1=ALU.max
                    )

                    pending_out = (
                        bass.AP(
                            y,
                            b * ybat + (row_base // 4) * xrow,
                            [[Jt * D, 128], [1, Jt * D]],
                        ),
                        R[:, :],
                    )
            nc.gpsimd.dma_start(out=pending_out[0], in_=pending_out[1])
    return nc


def kernel(x: np.ndarray, w: np.ndarray, b: np.ndarray) -> np.ndarray:
    global LAST_RESULT
    w = np.asarray(w, dtype=np.float32)
    bb = np.asarray(b, dtype=np.float32)
    key = (float(w[0]), float(w[1]), float(w[2]), float(bb[0]))
    if key not in _cache:
        _cache[key] = _build(*key)
    nc = _cache[key]

    x = np.asarray(x, dtype=np.float32)
    xpad = np.zeros((B, L + 3, D), dtype=np.float32)
    xpad[:, 3:, :] = x
    in_maps = [
        {"x": np.ascontiguousarray(xpad[c * BPC : (c + 1) * BPC])}
        for c in range(N_CORES)
    ]
    res = run_bass_kernel_spmd(nc, in_maps, core_ids=list(range(N_CORES)))
    LAST_RESULT = res
    return np.concatenate([r["y"] for r in res.results], axis=0)


# revision 29
# speedup vs baseline: 1.2644x; 1.2644x over previous
"""Trainium2 Bass kernel for nn_DistillingLayer: per-channel shared-weight
Conv1d(k=3, stride=2, pad=1) + ELU + MaxPool1d(k=3, stride=2, pad=1) over
x:(16, 4096, 512) f32.

Strategy
--------
- Data-parallel over batch: 8 cores x 2 batches each. No communication.
- Layout: L lives in the SBUF *free* dimension. Each partition owns S=16
  consecutive L-rows (times D=512 channels) plus a 3-row halo, so the whole
  conv+pool dataflow is per-partition local (DVE lanes cannot cross
  partitions).
- ELU is monotonic, so maxpool commutes with it: pool the *pre-activation*
  conv outputs, then apply ELU once on the pooled result (half the L-rows).
  The conv bias folds into the conv pass itself.
- Conv outputs are computed de-interleaved (ce[m]=c[2m], co[m]=c[2m+1]) so
  every elementwise op runs on contiguous or simply-strided views:
      c[i] = w0*x[2i-1] + w1*x[2i] + w2*x[2i+1] + b
      out[m] = elu(max(co[m-1], ce[m], co[m]))
- ELU(v) = max(v, exp(min(v, 0)) - 1); min via relu(-v) on ScalarE with
  scale=-1, exp on ScalarE, final fused (e-1) max v on VectorE
  scalar_tensor_tensor.
- Weights/bias are baked as immediates (kernel() receives concrete values).
"""

import os
import sys

import numpy as np

for _p in ("/opt/trn_rl_repo", "/root/.axon_site/_ro/trn_rl_repo"):
    if os.path.isdir(_p) and _p not in sys.path:
        sys.path.append(_p)

import json as _json

import concourse.bass as bass
import concourse.bass2jax as bass2jax
import concourse.bass_utils as bass_utils
import concourse.mybir as mybir
from concourse.bass_utils import run_bass_kernel_spmd
from concourse.tile import TileContext

# ---------------------------------------------------------------------------
# Workaround: this container's walrus build rejects instructions carrying more
# than one sync wait ("Too many sync wait commands" in setupSyncWait). Tile's
# scheduler freely attaches several waits to one instruction, so post-process
# the BIR JSON before compile: hoist all but the last wait onto same-engine
# NoOps inserted just before the instruction (per-engine program order makes
# sequential waits equivalent to a multi-wait).
# ---------------------------------------------------------------------------

_orig_compile_bir_kernel = bass_utils.compile_bir_kernel


def _split_multi_waits(bir_json: bytes) -> bytes:
    j = _json.loads(bir_json)
    ctr = 0
    changed = False
    for fn in j["functions"]:
        for bb in fn["blocks"]:
            out = []
            for ins in bb["instructions"]:
                si = ins.get("sync_info")
                waits = (si.get("on_wait") or []) if si else []
                if len(waits) > 1:
                    changed = True
                    for w in waits[:-1]:
                        ctr += 1
                        out.append(
                            {
                                "debug": ins.get("debug", 0),
                                "engine": ins["engine"],
                                "ins": [],
                                "outs": [],
                                "name": f"waitsplit-{ctr}",
                                "opcode": "NoOp",
                                "text_hint": "waitsplit",
                                "sync_info": {"on_update": [], "on_wait": [w]},
                            }
                        )
                    si["on_wait"] = [waits[-1]]
                out.append(ins)
            bb["instructions"] = out
    if not changed:
        return bir_json
    return _json.dumps(j).encode()


def _patched_compile_bir_kernel(bir_json, tmpdir, neff_name="file.neff"):
    return _orig_compile_bir_kernel(_split_multi_waits(bir_json), tmpdir, neff_name)


bass_utils.compile_bir_kernel = _patched_compile_bir_kernel
bass2jax.compile_bir_kernel = _patched_compile_bir_kernel

# Event-semaphore barriers cost ~3us each on cayman; the sequencer-level
# sem-only variant is ~10x cheaper. All barrier uses here (const-AP preamble,
# TileContext exit) already wait on the relevant completion semaphores, so the
# per-engine drains the full barrier adds are redundant.
_orig_barrier = bass.Bass.all_engine_barrier


def _fast_barrier(self, *, sem_only=False):
    return _orig_barrier(self, sem_only=True)


bass.Bass.all_engine_barrier = _fast_barrier

# ---------------------------------------------------------------------------

N_CORES = 8
B, L, D = 16, 4096, 512
BPC = B // N_CORES  # batches per core
S = 16              # L-rows per partition per tile
T = L // (128 * S)  # tiles per batch
LC = L // 2         # conv output length
LP = LC // 2        # pool output length
JT = S // 4         # pool-output rows per partition per tile

F32 = mybir.dt.float32
ALU = mybir.AluOpType
AF = mybir.ActivationFunctionType

_cache: dict = {}

# Exposed for test harnesses: the BassKernelResults of the last run.
LAST_RESULT = None


def _build(w0: float, w1: float, w2: float, bias: float) -> bass.Bass:
    nc = bass.Bass()
    # x is host-padded with 3 zero rows at the front of L: padded row r
    # holds true row r-3. This makes every tile a uniform full-128-partition
    # DMA (SWDGE descriptor fan-out degenerates to 1-2 SDMA engines for
    # partition counts != 128) and provides the conv zero-padding for free.
    x = nc.dram_tensor("x", [BPC, L + 3, D], F32, kind="ExternalInput")
    y = nc.dram_tensor("y", [BPC, LP, D], F32, kind="ExternalOutput")

    xrow = D              # elements per L-row
    xbat = (L + 3) * D    # elements per (padded) input batch
    ybat = LP * D

    # Variable tile schedule per batch: small tiles at the global start
    # (fast pipeline fill) and end (short drain tail), big in the middle.
    # Each entry is (row_base, S_t); rows covered = 128 * S_t.
    sched0 = [(0, 16), (2048, 16)]
    sched1 = sched0

    with TileContext(nc) as tc:
        with (
            tc.tile_pool(name="xp", bufs=3) as xp,
            tc.tile_pool(name="yp", bufs=2) as yp,
            tc.tile_pool(name="pp", bufs=2) as pp,
            tc.tile_pool(name="rp", bufs=2) as rp,
        ):
            # Output DMAs are emitted one tile LATE on the gpsimd queue so
            # their wait-for-compute is already satisfied when the trigger
            # executes and never gates the next tile's input DMA trigger
            # (engine DMA queues execute waits in program order).
            pending_out = None
            tiles = [(b, rb, st)
                     for b in range(BPC)
                     for rb, st in (sched0 if b == 0 else sched1)]
            for b, row_base, St in tiles:
                    Q = St // 2 + 1   # conv rows per partition (with halo)
                    Jt = St // 4      # pool-output rows per partition
                    # X layout per partition: 3 halo rows then S own rows.
                    # HBM supplies only the S own rows (plus 6KB for
                    # partition 0's halo, which reads the host pad zeros at
                    # row_base=0); partitions 1..127 get their halo from
                    # partition p-1's last rows via two SBUF->SBUF shifted
                    # DMAs (64+63 partitions — 127-partition DMAs degenerate
                    # to one SDMA engine, <=64 fan out fine).
                    X = xp.tile([128, (St + 3) * D], F32)
                    nc.gpsimd.dma_start(
                        out=X[:, :],
                        in_=bass.AP(
                            x,
                            b * xbat + row_base * xrow,
                            [[St * xrow, 128], [1, (St + 3) * xrow]],
                        ),
                    )
                    if pending_out is not None:
                        nc.gpsimd.dma_start(out=pending_out[0], in_=pending_out[1])
                        pending_out = None

                    Xv = X[:, :].rearrange("p (r d) -> p r d", d=D)
                    # conv row q (local) = c[2*O0 - 1 + q]; taps are x rows
                    # (local) 2q, 2q+1, 2q+2
                    ya = Xv[:, 0 : St + 1 : 2, :]
                    yb = Xv[:, 1 : St + 2 : 2, :]
                    yc = Xv[:, 2 : St + 3 : 2, :]

                    Y = yp.tile([128, Q * D], F32)
                    y3 = Y[:, :].rearrange("p (q d) -> p q d", d=D)

                    # conv (bias folded in): c = w0*ya + w1*yb + w2*yc + bias
                    nc.scalar.activation(y3, ya, AF.Copy, bias=bias, scale=w0)
                    nc.vector.scalar_tensor_tensor(
                        y3, yb, w1, y3, op0=ALU.mult, op1=ALU.add
                    )
                    nc.vector.scalar_tensor_tensor(
                        y3, yc, w2, y3, op0=ALU.mult, op1=ALU.add
                    )
                    if row_base == 0:
                        # left pool pad: c[-1] = -inf (partition 0 only)
                        nc.vector.memset(Y[0:1, 0:D], float("-inf"))

                    # maxpool (pre-activation; ELU is monotonic):
                    # out[jl] = max(Y[2jl], Y[2jl+1], Y[2jl+2])
                    P = pp.tile([128, Jt * D], F32)
                    p3 = P[:, :].rearrange("p (j d) -> p j d", d=D)
                    nc.vector.tensor_tensor(
                        p3,
                        y3[:, 0 : 2 * Jt - 1 : 2, :],
                        y3[:, 1 : 2 * Jt : 2, :],
                        op=ALU.max,
                    )
                    nc.vector.tensor_tensor(
                        p3, p3, y3[:, 2 : 2 * Jt + 1 : 2, :], op=ALU.max
                    )

                    # ELU(v) = max(v, exp(min(v,0)) - 1)
                    R = rp.tile([128, Jt * D], F32)
                    nc.scalar.activation(R[:, :], P[:, :], AF.Relu, scale=-1.0)
                    nc.scalar.activation(R[:, :], R[:, :], AF.Exp, scale=-1.0)
                    nc.vector.scalar_tensor_tensor(
                        R[:, :], R[:, :], -1.0, P[:, :], op0=ALU.add, op1=ALU.max
                    )

                    pending_out = (
                        bass.AP(
                            y,
                            b * ybat + (row_base // 4) * xrow,
                            [[Jt * D, 128], [1, Jt * D]],
                        ),
                        R[:, :],
                    )
            nc.gpsimd.dma_start(out=pending_out[0], in_=pending_out[1])
    return nc


def kernel(x: np.ndarray, w: np.ndarray, b: np.ndarray) -> np.ndarray:
    global LAST_RESULT
    w = np.asarray(w, dtype=np.float32)
    bb = np.asarray(b, dtype=np.float32)
    key = (float(w[0]), float(w[1]), float(w[2]), float(bb[0]))
    if key not in _cache:
        _cache[key] = _build(*key)
    nc = _cache[key]

    x = np.asarray(x, dtype=np.float32)
    xpad = np.zeros((B, L + 3, D), dtype=np.float32)
    xpad[:, 3:, :] = x
    in_maps = [
        {"x": np.ascontiguousarray(xpad[c * BPC : (c + 1) * BPC])}
        for c in range(N_CORES)
    ]
    res = run_bass_kernel_spmd(nc, in_maps, core_ids=list(range(N_CORES)))
    LAST_RESULT = res
    return np.concatenate([r["y"] for r in res.results], axis=0)
